# revision 22
# baseline (speedup 1.0000x reference)
"""MultiHeadAttention TRN2 Bass kernel.

Full-input contract: kernel(**inputs) takes the unsharded tensors from
setup_inputs() and returns the full [4, 2048, 512] output.

Sharding: 8 cores = 4 batches x 2 query-halves. Each core computes its own
[1024, 512] slice of the output for one batch over all 8 heads, so the
gather is a pure concatenation (no collectives, no all-reduce).

Per-core pipeline (bf16 matmul inputs, fp32 psum accumulate):
  1. Weights DMA on the gpsimd queue (concurrent with x on sync), cast to
     bf16 on ACT, PE-transposed.  K then Q chunks: cast, PE-transpose,
     project; KT / zero-padded per-head QTz written by ACT (Identity+bias)
     so DVE doesn't pace the projection phase.
  2. Head-0 scores+exp start right after K+Q; the V projection and its
     x-transposes are interleaved into head-0's window on the PE chain
     (exp on ACT is the long pole, started ~45us earlier than a serial
     phase ordering).
  3. Scores use full-height 128x128 stationary tiles (QTz zero-padding)
     -- partial-height matmuls don't register as PE activity in the HAM
     clock gate and leave the PE at 1.2 GHz.  One [128,1024] exp per
     k-chunk.  E tiles live in a 22-slot ring (saves 20KB SBUF vs
     2 full-head buffers).
  4. finalize(h): OU copy + rowsum reciprocal (two [1,512] halves) on DVE
     inside the attention phase; when a head pair completes, its rowsum
     broadcast (tiny matmul at 32-aligned partitions) and normalize
     multiply run during attention too.  Tail only handles pair 3 +
     out-projection.
"""
import contextlib

import numpy as np

import bass_rust
import concourse.bass as bass
import concourse.mybir as mybir
import concourse.tile as tile
from concourse.bass_utils import run_bass_kernel_spmd
from concourse.masks import make_identity
from concourse.tile import add_dep_helper

F32 = mybir.dt.float32
F32R = mybir.dt.float32r
BF16 = mybir.dt.bfloat16

B, S, D_MODEL = 4, 2048, 512
NUM_HEADS = 8
HEAD_DIM = 64
SQ = S // 2  # queries per core
N_CORES = 8
SCALE = 1.0 / 8.0  # 1/sqrt(HEAD_DIM)

_split_ctr = [0]


def split_waits(nc, max_waits: int = 1):
    """walrus codegen rejects instructions carrying >1 sync wait; move the
    extras onto standalone EventSemaphore instructions on the same engine."""
    for f in nc.m.functions:
        for blk in f.blocks:
            new_insts = []
            changed = False
            for inst in blk.instructions:
                si = inst.sync_info
                if si is not None and si.on_wait and len(si.on_wait) > max_waits:
                    waits = list(si.on_wait)
                    extra, keep = waits[:-max_waits], waits[-max_waits:]
                    for w in extra:
                        _split_ctr[0] += 1
                        ev = mybir.InstEventSemaphore(
                            name=f"I-wsplit-{_split_ctr[0]}", ins=[], outs=[]
                        )
                        ev.engine = inst.engine
                        ev.sync_info = bass_rust.SyncInfo(on_wait=[w], on_update=[])
                        new_insts.append(ev)
                    inst.sync_info = bass_rust.SyncInfo(
                        on_wait=keep, on_update=list(si.on_update)
                    )
                    changed = True
                new_insts.append(inst)
            if changed:
                blk.instructions = new_insts


def build_mha():
    nc = bass.Bass("TRN2", target_bir_lowering=False, debug=False, num_devices=1)

    qd = nc.declare_dram_parameter("q", [SQ, D_MODEL], F32, isOutput=False).ap()
    kd = nc.declare_dram_parameter("k", [S, D_MODEL], F32, isOutput=False).ap()
    vd = nc.declare_dram_parameter("v", [S, D_MODEL], F32, isOutput=False).ap()
    wts = {
        n: nc.declare_dram_parameter(n, [D_MODEL, D_MODEL], F32, isOutput=False).ap()
        for n in ("wq", "wk", "wv", "wo")
    }
    bias = {
        n: nc.declare_dram_parameter(n, [D_MODEL], F32, isOutput=False).ap()
        for n in ("bq", "bk", "bv", "bo")
    }
    outd = nc.declare_dram_parameter("out", [SQ, D_MODEL], F32, isOutput=True).ap()

    H2 = NUM_HEADS // 2  # head pairs = dout tiles of 128
    KTILES = S // 128  # 16
    EH_SLOTS = 20  # ring depth for E tiles (max ~18 live)

    with tile.TileContext(nc) as tc, contextlib.ExitStack() as top:
        consts = top.enter_context(tc.tile_pool(name="consts", bufs=1))
        wt_pool = top.enter_context(tc.tile_pool(name="wt", bufs=1))
        proj_out = top.enter_context(tc.tile_pool(name="proj_out", bufs=1))
        epilog = top.enter_context(tc.tile_pool(name="epilog", bufs=1))
        ehpool = top.enter_context(tc.tile_pool(name="ehpool", bufs=EH_SLOTS))
        # scores psum gets banks 0-3 for the whole kernel; projection psum
        # (banks 4-7) frees before the AV psum pool opens
        ps_s = top.enter_context(tc.tile_pool(name="ps_s", bufs=2, space="PSUM"))

        # ---- weight DMAs go FIRST on the gpsimd queue (everything else on
        # it is needed much later; the first PE transposes wait on wq/wk)
        w_nats = {}
        ld_w = top.enter_context(tc.tile_pool(name="ld_w", bufs=2))
        for name in ("wq", "wk"):
            w_nats[name] = ld_w.tile(
                [128, 4, D_MODEL], F32, name=f"wnat_{name}", tag="wnat"
            )
            nc.gpsimd.dma_start(
                out=w_nats[name], in_=wts[name].rearrange("(c p) m -> p c m", p=128)
            )

        # ---- constants
        ident = consts.tile([128, 128], F32)
        make_identity(nc, ident)
        identb = consts.tile([128, 128], BF16)
        nc.vector.tensor_copy(identb, ident)
        # per-partition bias tiles for Q/K (bias indexed by d_out partition)
        bqt = consts.tile([128, 4], F32)
        bkt = consts.tile([128, 4], F32)
        for t_, name in ((bqt, "bq"), (bkt, "bk")):
            nc.gpsimd.dma_start(
                out=t_, in_=bias[name].rearrange("(c p) -> p c", p=128)
            )
        # free-dim broadcast biases for V / out
        bvb = consts.tile([128, D_MODEL], F32)
        bob = consts.tile([128, D_MODEL], F32)
        for t_, name in ((bvb, "bv"), (bob, "bo")):
            src = bias[name]
            nc.gpsimd.dma_start(
                out=t_,
                in_=bass.AP(tensor=src.tensor, offset=src.offset, ap=[[0, 128], [1, D_MODEL]]),
            )
        ones8 = consts.tile([128, NUM_HEADS], BF16)
        nc.vector.memset(ones8, 1.0)
        # upper/lower-half selection rows for the rowsum broadcast matmuls
        eud_np = np.zeros((2, 128), np.float32)
        eud_np[0, 0:HEAD_DIM] = 1.0
        eud_np[1, HEAD_DIM:128] = 1.0
        eud_dram = nc.inline_tensor(eud_np, name="eud_const")
        e_up_f = consts.tile([1, 128], F32)
        e_dn_f = consts.tile([1, 128], F32)
        nc.gpsimd.dma_start(out=e_up_f, in_=eud_dram.ap()[0:1, :])
        nc.gpsimd.dma_start(out=e_dn_f, in_=eud_dram.ap()[1:2, :])
        e_up = consts.tile([1, 128], BF16)
        e_dn = consts.tile([1, 128], BF16)
        nc.vector.tensor_copy(e_up, e_up_f)
        nc.vector.tensor_copy(e_dn, e_dn_f)

        # ---- long-lived activation tiles
        WT = {
            "wo": [
                wt_pool.tile([128, D_MODEL], BF16, name=f"wt_wo_{dc}", tag=f"wt_wo_{dc}")
                for dc in range(4)
            ]
        }
        # per-head zero-padded Q^T: head h occupies partitions
        # (h%2)*64..(h%2)*64+63, other half is zero
        QTz = [
            proj_out.tile([128, SQ], BF16, name=f"qtz_{h}", tag=f"qtz_{h}")
            for h in range(NUM_HEADS)
        ]
        KT = [proj_out.tile([128, S], BF16, name=f"kt_{t}", tag=f"kt_{t}") for t in range(H2)]
        V = [
            proj_out.tile([128, NUM_HEADS, HEAD_DIM + 1], BF16, name=f"v_{sc}", tag=f"v_{sc}")
            for sc in range(KTILES)
        ]
        OU = [epilog.tile([128, SQ], F32, name=f"ou_{t}", tag=f"ou_{t}") for t in range(H2)]
        OMT = [epilog.tile([128, SQ], BF16, name=f"omt_{t}", tag=f"omt_{t}") for t in range(H2)]
        # per-head rowsum reciprocals (partition 0 -- nonzero partition
        # bases fail BIR verification for 1-partition accesses)
        RSR = [
            epilog.tile([1, SQ], BF16, name=f"rsr_{h}", tag=f"rsr_{h}")
            for h in range(NUM_HEADS)
        ]

        for h in range(NUM_HEADS):
            nc.gpsimd.memset(QTz[h], 0.0)

        pe_chain = [None]

        def chain(bi):
            if pe_chain[0] is not None:
                add_dep_helper(bi.ins, pe_chain[0].ins, reason="pe-order")
            pe_chain[0] = bi

        # E-tile ring: slot per (head, kc)
        eh_slots = {}

        def eh_slot(h, kc):
            key = (h, kc)
            if key not in eh_slots:
                eh_slots[key] = ehpool.tile(
                    [128, SQ], BF16, name=f"eh_{h}_{kc}", tag="eh"
                )
            return eh_slots[key]

        def emit_scores(h):
            """32 score matmuls + 16 exps for head h; returns nothing
            (E lands in the slot ring)."""
            t = h // 2
            for kb in range(KTILES // 2):
                pss = []
                for j in range(2):
                    kc = 2 * kb + j
                    pscore = ps_s.tile([128, SQ], F32, tag="pscore")
                    for qc in range(SQ // 512):
                        sl = slice(qc * 512, (qc + 1) * 512)
                        chain(
                            nc.tensor.matmul(
                                pscore[:, sl],
                                KT[t][:, kc * 128 : (kc + 1) * 128],
                                QTz[h][:, sl],
                                start=True,
                                stop=True,
                            )
                        )
                    pss.append((kc, pscore))
                for kc, pscore in pss:
                    nc.scalar.activation(
                        eh_slot(h, kc),
                        pscore,
                        mybir.ActivationFunctionType.Exp,
                        scale=SCALE,
                    )
                yield kb

        def emit_av(h, po, kb):
            for j in range(2):
                kc = 2 * kb + j
                peh = eh_slot(h, kc)
                for qc in range(SQ // 512):
                    sl = slice(qc * 512, (qc + 1) * 512)
                    chain(
                        nc.tensor.matmul(
                            po[:, sl],
                            V[kc][:, h, :],
                            peh[:, sl],
                            start=(kc == 0),
                            stop=(kc == KTILES - 1),
                        )
                    )

        def finalize(h, po):
            t, half = h // 2, h % 2
            # first reciprocal quarter before the OU copy: the tail's
            # broadcast matmul waits only ~1.7us instead of ~8us
            with nc.allow_low_precision("softmax denominators in bf16"):
                for qq in range(4):
                    sl = slice(qq * 256, (qq + 1) * 256)
                    nc.vector.reciprocal(
                        RSR[h][:, sl], po[HEAD_DIM : HEAD_DIM + 1, sl]
                    )
                    if qq == 0:
                        nc.vector.tensor_copy(
                            OU[t][half * HEAD_DIM : (half + 1) * HEAD_DIM, :],
                            po[0:HEAD_DIM, :],
                        )
            # drop consumed E slots for head h (frees ring bookkeeping)
            for kc in range(KTILES):
                eh_slots.pop((h, kc), None)

        def pair_normalize(t):
            """Broadcast rowsum reciprocals for pair t and normalize into
            OMT[t].  pr shares the score-psum ring."""
            pr = ps_s.tile([128, SQ], F32, tag="pscore")
            for qc in range(2):
                sl = slice(qc * 512, (qc + 1) * 512)
                chain(nc.tensor.matmul(pr[:, sl], e_up, RSR[2 * t][:, sl], start=True, stop=False))
                chain(nc.tensor.matmul(pr[:, sl], e_dn, RSR[2 * t + 1][:, sl], start=False, stop=True))
            nc.vector.tensor_mul(OMT[t], OU[t], pr)

        # ================= phase A: weights, K/Q proj, head 0 =============
        with (
            tc.tile_pool(name="ld", bufs=2) as ld_pool,
            tc.tile_pool(name="xt", bufs=2) as xt_pool,
            tc.tile_pool(name="wtq", bufs=1) as wtq_pool,
            tc.tile_pool(name="pp", bufs=2, space="PSUM") as pp,
        ):
            for n in ("wq", "wk", "wv"):
                WT[n] = [
                    wtq_pool.tile(
                        [128, D_MODEL], BF16, name=f"wt_{n}_{dc}", tag=f"wt_{n}_{dc}"
                    )
                    for dc in range(4)
                ]

            def load_weight(name):
                """Cast a landed weight to bf16 and PE-transpose into WT.
                wq/wk were DMA'd at kernel start; wv/wo DMA on demand from
                the same 2-slot ring."""
                w_nat = w_nats.get(name)
                if w_nat is None:
                    w_nat = ld_w.tile(
                        [128, 4, D_MODEL], F32, name=f"wnat_{name}", tag="wnat"
                    )
                    nc.gpsimd.dma_start(
                        out=w_nat, in_=wts[name].rearrange("(c p) m -> p c m", p=128)
                    )
                w_r = ld_pool.tile([128, 4, D_MODEL], BF16, name=f"wr_{name}", tag="ldr")
                nc.scalar.copy(w_r, w_nat)
                for oc in range(4):  # d_out chunk of W natural
                    pt = pp.tile([128, D_MODEL], BF16, tag="ptrans")
                    for dc in range(4):
                        chain(
                            nc.tensor.transpose(
                                pt[:, dc * 128 : (dc + 1) * 128],
                                w_r[:, oc, dc * 128 : (dc + 1) * 128],
                                identb,
                            )
                        )
                    for dc in range(4):
                        nc.vector.tensor_copy(
                            WT[name][dc][:, oc * 128 : (oc + 1) * 128],
                            pt[:, dc * 128 : (dc + 1) * 128],
                        )


            def transpose_chunk(src_ap, s0, cast_engine="scalar"):
                """DMA + cast + PE-transpose a [512, 512] chunk of src;
                returns the x^T tile."""
                xt_c = xt_pool.tile([128, 4, 512], BF16, tag="xt")
                x_nat = ld_pool.tile([128, 4, D_MODEL], F32, tag="ldraw", bufs=3)
                nc.sync.dma_start(
                    out=x_nat,
                    in_=src_ap[s0 : s0 + 512, :].rearrange("(c p) m -> p c m", p=128),
                )
                x_r = ld_pool.tile([128, 4, D_MODEL], BF16, tag="ldr")
                if cast_engine == "scalar":
                    nc.scalar.copy(x_r, x_nat)
                elif cast_engine == "gpsimd":
                    nc.gpsimd.tensor_copy(x_r, x_nat)
                else:
                    nc.vector.tensor_copy(x_r, x_nat)
                for st in range(4):
                    pt = pp.tile([128, D_MODEL], BF16, tag="ptrans")
                    for dc in range(4):
                        chain(
                            nc.tensor.transpose(
                                pt[:, dc * 128 : (dc + 1) * 128],
                                x_r[:, st, dc * 128 : (dc + 1) * 128],
                                identb,
                            )
                        )
                    nc.vector.tensor_copy(
                        xt_c[:, :, st * 128 : (st + 1) * 128],
                        pt.rearrange("p (c f) -> p c f", c=4),
                    )
                return xt_c

            def k_proj(c, xt_c, aux=False):
                for t in range(H2):
                    pj = pp.tile([128, 512], F32, tag="pproj")
                    for dc in range(4):
                        chain(
                            nc.tensor.matmul(
                                pj,
                                WT["wk"][dc][:, t * 128 : (t + 1) * 128],
                                xt_c[:, dc, :],
                                start=(dc == 0),
                                stop=(dc == 3),
                            )
                        )
                    if aux:
                        nc.vector.tensor_scalar_add(
                            KT[t][:, c * 512 : (c + 1) * 512],
                            pj,
                            bkt[:, t : t + 1],
                        )
                    else:
                        nc.scalar.activation(
                            KT[t][:, c * 512 : (c + 1) * 512],
                            pj,
                            mybir.ActivationFunctionType.Identity,
                            bias=bkt[:, t : t + 1],
                        )

            def k_chunk(c, aux=False):
                """aux=True: cast + KT writes on gpsimd so they never sit in
                ACT's FIFO ahead of head-0 exps (head-of-line blocking)."""
                k_proj(c, transpose_chunk(kd, c * 512), aux=aux)

            def q_chunk(c):
                xt_c = transpose_chunk(qd, c * 512)
                for t in range(H2):
                    pj = pp.tile([128, 512], F32, tag="pproj")
                    for dc in range(4):
                        chain(
                            nc.tensor.matmul(
                                pj,
                                WT["wq"][dc][:, t * 128 : (t + 1) * 128],
                                xt_c[:, dc, :],
                                start=(dc == 0),
                                stop=(dc == 3),
                            )
                        )
                    sl = slice(c * 512, (c + 1) * 512)
                    nc.scalar.activation(
                        QTz[2 * t][0:HEAD_DIM, sl],
                        pj[0:HEAD_DIM, :],
                        mybir.ActivationFunctionType.Identity,
                        bias=bqt[0:HEAD_DIM, t : t + 1],
                    )
                    nc.scalar.activation(
                        QTz[2 * t + 1][HEAD_DIM:128, sl],
                        pj[HEAD_DIM:128, :],
                        mybir.ActivationFunctionType.Identity,
                        bias=bqt[HEAD_DIM:128, t : t + 1],
                    )

            # minimum prefix for head-0 scores: first k chunk + both q chunks
            # (k0's transposes give the PE work ~5us before the weight
            # casts land; its projection waits on wk anyway)
            xt_k0 = transpose_chunk(kd, 0)
            load_weight("wq")
            load_weight("wk")
            k_proj(0, xt_k0)
            q_chunk(0)
            q_chunk(1)

            # remaining projection work, interleaved into head-0's score
            # loop (exp on ACT is the pacer; PE fills its slack)
            def v_unit_pair(c):
                state = {}

                def unit1(c=c):
                    state["xt"] = transpose_chunk(vd, c * 512, cast_engine="vector")

                def unit2(c=c):
                    xt_c = state["xt"]
                    for st in range(4):
                        sc = c * 4 + st
                        pj = pp.tile([128, 512], F32, tag="pproj")
                        for dc in range(4):
                            chain(
                                nc.tensor.matmul(
                                    pj,
                                    xt_c[:, dc, st * 128 : (st + 1) * 128],
                                    WT["wv"][dc],
                                    start=(dc == 0),
                                    stop=(dc == 3),
                                )
                            )
                        pj3 = pj.rearrange("p (h d) -> p h d", h=NUM_HEADS)
                        nc.vector.tensor_add(
                            V[sc][:, :, 0:HEAD_DIM],
                            pj3,
                            bvb.rearrange("p (h d) -> p h d", h=NUM_HEADS),
                        )
                        nc.vector.tensor_copy(
                            V[sc][:, :, HEAD_DIM : HEAD_DIM + 1],
                            ones8.rearrange("p (h o) -> p h o", o=1),
                        )

                return [unit1, unit2]

            units = [lambda: k_chunk(1, aux=True), lambda: k_chunk(2, aux=True),
                     lambda: k_chunk(3, aux=True), lambda: load_weight("wv")]
            for c in range(S // 512):
                units.extend(v_unit_pair(c))

            for kb in emit_scores(0):
                if units:
                    units.pop(0)()
            while units:
                units.pop(0)()
            load_weight("wo")

        # ================= phase B: heads 1..7 ============================
        with tc.tile_pool(name="ps_o", bufs=2, space="PSUM") as ps_o:
            po_prev = ps_o.tile([HEAD_DIM + 1, SQ], F32, tag="po")  # head 0
            for h in range(1, NUM_HEADS):
                po_cur = ps_o.tile([HEAD_DIM + 1, SQ], F32, tag="po")
                prev_h = h - 1
                for kb in emit_scores(h):
                    if h == NUM_HEADS - 1:
                        # head 6's AV remainder, then its finalize as soon
                        # as po(6) completes: its reciprocals run on DVE
                        # during head 7 instead of stacking in the tail
                        if kb < 2:
                            emit_av(prev_h, po_prev, kb + 6)
                        if kb == 2:
                            finalize(prev_h, po_prev)
                        if kb >= 2:
                            emit_av(h, po_cur, kb - 2)
                    else:
                        emit_av(prev_h, po_prev, kb)
                        # heads 6 and 7 run their own AV at a 2-batch lag
                        if h == NUM_HEADS - 2 and kb >= 2:
                            emit_av(h, po_cur, kb - 2)
                    # normalize pair t one full head after its reciprocals
                    # were kicked off, so the PE chain never waits on DVE
                    if kb == 5 and h % 2 == 1 and h >= 3:
                        pair_normalize((h - 3) // 2)
                if h != NUM_HEADS - 1:
                    finalize(prev_h, po_prev)
                po_prev = po_cur
            for kb in range(KTILES // 2 - 2, KTILES // 2):
                emit_av(NUM_HEADS - 1, po_prev, kb)
            finalize(NUM_HEADS - 1, po_prev)

        # ============= tail: pair-3 normalize + out projection ============
        with (
            tc.tile_pool(name="outsb", bufs=3) as outsb,
            tc.tile_pool(name="ps_f", bufs=2, space="PSUM") as ps_f,
        ):
            t3 = H2 - 1
            pr = ps_s.tile([128, SQ], F32, tag="pscore")
            for qq in range(4):
                sl = slice(qq * 256, (qq + 1) * 256)
                chain(nc.tensor.matmul(pr[:, sl], e_up, RSR[2 * t3][:, sl], start=True, stop=False))
                chain(nc.tensor.matmul(pr[:, sl], e_dn, RSR[2 * t3 + 1][:, sl], start=False, stop=True))
                nc.vector.tensor_mul(OMT[t3][:, sl], OU[t3][:, sl], pr[:, sl])
                for sq in range(qq * 2, qq * 2 + 2):
                    pf = ps_f.tile([128, D_MODEL], F32, tag="pf")
                    for t in range(H2):
                        chain(
                            nc.tensor.matmul(
                                pf,
                                OMT[t][:, sq * 128 : (sq + 1) * 128],
                                WT["wo"][t],
                                start=(t == 0),
                                stop=(t == H2 - 1),
                            )
                        )
                    ot = outsb.tile([128, D_MODEL], F32, tag="ot")
                    nc.vector.tensor_add(ot, pf, bob)
                    nc.sync.dma_start(out=outd[sq * 128 : (sq + 1) * 128, :], in_=ot)

    split_waits(nc)
    return nc


_cached_nc = None


def _get_nc():
    global _cached_nc
    if _cached_nc is None:
        _cached_nc = build_mha()
    return _cached_nc


def kernel(q, k, v, mask, Wq, bq, Wk, bk, Wv, bv, Wo, bo, **_unused):
    q = np.asarray(q, dtype=np.float32)
    k = np.asarray(k, dtype=np.float32)
    v = np.asarray(v, dtype=np.float32)
    weights = {
        "wq": np.ascontiguousarray(np.asarray(Wq, np.float32)),
        "wk": np.ascontiguousarray(np.asarray(Wk, np.float32)),
        "wv": np.ascontiguousarray(np.asarray(Wv, np.float32)),
        "wo": np.ascontiguousarray(np.asarray(Wo, np.float32)),
        "bq": np.ascontiguousarray(np.asarray(bq, np.float32)),
        "bk": np.ascontiguousarray(np.asarray(bk, np.float32)),
        "bv": np.ascontiguousarray(np.asarray(bv, np.float32)),
        "bo": np.ascontiguousarray(np.asarray(bo, np.float32)),
    }
    in_maps = []
    for core in range(N_CORES):
        b, qh = core // 2, core % 2
        in_maps.append(
            {
                "q": np.ascontiguousarray(q[b, qh * SQ : (qh + 1) * SQ, :]),
                "k": np.ascontiguousarray(k[b]),
                "v": np.ascontiguousarray(v[b]),
                **weights,
            }
        )
    nc = _get_nc()
    res = run_bass_kernel_spmd(nc, in_maps, list(range(N_CORES)))
    out = np.empty((B, S, D_MODEL), dtype=np.float32)
    for core in range(N_CORES):
        b, qh = core // 2, core % 2
        out[b, qh * SQ : (qh + 1) * SQ, :] = res.results[core]["out"]
    return out
